# revision 77
# baseline (speedup 1.0000x reference)
"""Multi-head attention with relative position bias (music-transformer skew)
on 8 Trainium2 NeuronCores.

Sharding: batch x head-group. Core c handles batch b = c//4 and heads
4*(c%4) .. 4*(c%4)+3 (tensor-parallel split of the 16 heads / W column dims,
Wo row-parallel). Partial outputs are summed on the host (the all-reduce
equivalent), bias added on the host.

Per-core pipeline (bf16 matmuls, fp32 PSUM accumulation):
  x^T via PE transposes -> Q^T,K^T ([col,n]) and V ([m,col]) projections.
  Per head pair, one software-pipelined loop over row blocks i:
    expR = exp((Q/8) @ E1^T) -> DRAM (exp fused into the mandatory
    PSUM->SBUF copy) as ONE flat DMA per (i,hs) whose per-partition tail
    spills into the next DRAM row's low columns (the filler the skewed
    read needs); skew-read back (stride N-1 access pattern) as relE with
    the causal diagonal pre-masked by a 0/1 triangle;
    expS = exp((Q/8) @ K^T) straight out of PSUM;
    P = expS * relE with row sums Z fused into the same DVE op;
    P^T = P.T @ diag(1/Z) on the PE (normalization for free), delayed two
    iterations so the PE never stalls on the softmax chain;
  then O^T accumulates V-side (two heads in complementary PE column groups)
  and y = O^T.T @ Wo (bf16 partials, summed on the host).
  Pair 1 walks its row blocks in DESCENDING order so the final softmax
  chains are the cheap short rows and the O^T/y work for its high row
  blocks overlaps the remaining short iterations.
"""

import numpy as np

import concourse.bass as bass
import concourse.tile as tile
from concourse import bacc, mybir
from concourse.bass import ds, ts
from concourse.bass_utils import run_bass_kernel_spmd
from concourse.masks import make_identity

B, N, D, H, DH, DO = 2, 1024, 1024, 16, 64, 1024
HPC = 4              # heads per core
COLS = HPC * DH      # 256 projection columns per core
NB = N // 128        # 8 row blocks
KT = D // 128        # 8 contraction tiles
F32 = mybir.dt.float32
BF16 = mybir.dt.bfloat16
SCALE = 1.0 / np.sqrt(DH)
EXP = mybir.ActivationFunctionType.Exp
MULT = mybir.AluOpType.mult


def _body(tc):
    nc = tc.nc
    xb = nc.dram_tensor("xb", [N, D], F32, kind="ExternalInput")
    wq = nc.dram_tensor("wq", [D, COLS], F32, kind="ExternalInput")
    wk = nc.dram_tensor("wk", [D, COLS], F32, kind="ExternalInput")
    wv = nc.dram_tensor("wv", [D, COLS], F32, kind="ExternalInput")
    wo = nc.dram_tensor("wo", [COLS, DO], F32, kind="ExternalInput")
    e1 = nc.dram_tensor("e1", [N, DH], F32, kind="ExternalInput")
    qes = [nc.dram_tensor(f"qes{h}", [N, N], BF16) for h in range(HPC)]

    from contextlib import ExitStack
    ctx = ExitStack()
    singles = ctx.enter_context(tc.tile_pool(name="singles", bufs=1))
    persist = ctx.enter_context(tc.tile_pool(name="persist", bufs=1))
    stage = ctx.enter_context(tc.tile_pool(name="stage", bufs=2))
    work = ctx.enter_context(tc.tile_pool(name="work", bufs=2))
    ptp = ctx.enter_context(tc.tile_pool(name="ptp", bufs=1))
    pss = ctx.enter_context(tc.tile_pool(name="pss", bufs=6, space="PSUM"))

    ident = singles.tile([128, 128], BF16, tag="ident", name="ident")
    make_identity(nc, ident)

    # ---- x: 8 chunked loads (Pool SWDGE), pipelined ACT casts, PE transposes.
    # Staging layout: xTc[c][:, k*128:(k+1)*128] = x^T k-tile of n-chunk c,
    # i.e. x[n-block c, :].T laid out k-major.  Consumers index per chunk.
    xTc = [persist.tile([128, D], BF16, tag=f"xTc{c}", name=f"xTc{c}")
           for c in range(NB)]

    # weight/E loads ride the DVE HWDGE queue; the wk issue deliberately
    # sits AFTER the wq-ct0 cast in the DVE queue so its transfer
    # interleaves behind the first x chunks on the (serial) DMA engines.
    wl_q = stage.tile([128, KT, COLS], F32, tag="ld", name="wldq", bufs=3)
    wqr = wq.rearrange("(kt p) c -> p kt c", p=128)
    nc.sync.dma_start(out=wl_q[:, :, 0:128], in_=wqr[:, :, 0:128])
    wq_bf = persist.tile([128, KT, COLS], BF16, tag="wqb", name="wqb")
    nc.vector.tensor_copy(wq_bf[:, :, 0:128], wl_q[:, :, 0:128])
    with tc.tile_wait_until(0.0100):
        nc.sync.dma_start(out=wl_q[:, :, 128:256], in_=wqr[:, :, 128:256])
    wl_k = stage.tile([128, KT, COLS], F32, tag="ld", name="wldk", bufs=3)
    el = stage.tile([128, 8, DH], F32, tag="lde", name="eld", bufs=1)

    # x chunks, e1 and wk all ride the Pool SWDGE queue: descriptor
    # generation is serial (~1us each), which naturally staggers their
    # transfer-start times into the intended interleave on the (serial)
    # DMA engines: wq0 | x0 | e1 | x1 x2 x3 | wk | x4..x7.  The late
    # chunks are pinned so the first qes writes / skew reads can slot in.
    xls = []
    xpin = {4: 0.0125, 5: 0.0140, 6: 0.0155, 7: 0.0170}
    for c in range(NB):
        xl = stage.tile([128, D], F32, tag="xld", name="xld", bufs=4)
        with tc.tile_wait_until(xpin.get(c, 0), enable=c in xpin):
            nc.gpsimd.dma_start(
                out=xl, in_=xb.rearrange("(nt p) d -> p nt d", p=128)[:, c, :])
        xls.append(xl)
        if c == 0:
            nc.gpsimd.dma_start(
                out=el, in_=e1.rearrange("(nt p) d -> p nt d", p=128))
        elif c == 1:
            nc.gpsimd.dma_start(
                out=wl_k, in_=wk.rearrange("(kt p) c -> p kt c", p=128))

    # 0/1 lower-triangular (causal keep) mask, bf16
    tri01 = singles.tile([128, 128], BF16, tag="tri01", name="tri01")
    nc.gpsimd.memset(tri01, 1.0)
    nc.gpsimd.affine_select(
        out=tri01, in_=tri01, compare_op=mybir.AluOpType.is_ge,
        fill=0.0, base=0, pattern=[[-1, 128]], channel_multiplier=1,
    )
    # additive causal mask: 0 on lower incl diag, -1e9 strictly above
    negtri = singles.tile([128, 128], BF16, tag="negtri", name="negtri")
    nc.gpsimd.memset(negtri, 0.0)
    nc.gpsimd.affine_select(
        out=negtri, in_=negtri, compare_op=mybir.AluOpType.is_ge,
        fill=-1e9, base=0, pattern=[[-1, 128]], channel_multiplier=1,
    )

    e1t = singles.tile([128, N], BF16, tag="e1t", name="e1t")

    def emit_e1t():
        # E1^T [64, 1024] bf16, duplicated to partitions 64..127
        ec = stage.tile([128, 8, DH], BF16, tag="ec", name="ec", bufs=1)
        nc.vector.tensor_copy(ec, el)
        for half in range(2):
            ps = pss.tile([64, 512], BF16, tag="sp", name="sp")
            for q in range(4):
                nc.tensor.transpose(
                    ps[:, ts(q, 128)], ec[:, 4 * half + q, :], ident)
            nc.vector.tensor_copy(e1t[0:64, ds(512 * half, 512)], ps)
        nc.sync.dma_start(out=e1t[64:128, :], in_=e1t[0:64, :])

    qt = [persist.tile([128, N], BF16, tag=f"qt{c}", name=f"qt{c}") for c in range(2)]
    kt_sb = [persist.tile([128, N], BF16, tag=f"kt{c}", name=f"kt{c}") for c in range(2)]
    wk_bf = persist.tile([128, KT, COLS], BF16, tag="wkb", name="wkb")

    # held PSUM banks (tag spj) for the streamed pair-0 projections; the
    # same two banks are reused by pv_pos later in the kernel.
    jq = [None, None]
    jk = [None, None]

    def cast_transpose_chunk(c):
        xc = stage.tile([128, D], BF16, tag="xcc", name="xcc", bufs=3)
        if c % 2 == 0:
            nc.vector.tensor_copy(xc, xls[c])
        else:
            nc.scalar.copy(xc, xls[c])
        for h in range(2):
            ps = pss.tile([128, 512], BF16, tag="sp", name="sp")
            for k in range(4):
                nc.tensor.transpose(
                    ps[:, ts(k, 128)], xc[:, ts(4 * h + k, 128)], ident)
            nc.vector.tensor_copy(xTc[c][:, ts(h, 512)], ps)

    def stream_q0(c):
        """Pair-0 Q projection column c, copied out (scaled) per chunk."""
        nh, cl = c // 4, c % 4
        if cl == 0:
            jq[nh] = pss.tile([128, 512], F32, tag="spj", name="spj", bufs=2)
        for k in range(KT):
            nc.tensor.matmul(
                jq[nh][:, ts(cl, 128)], wq_bf[:, k, ds(0, 128)],
                xTc[c][:, ts(k, 128)],
                start=(k == 0), stop=(k == KT - 1),
            )
        nc.vector.tensor_scalar_mul(
            qt[0][:, ts(c, 128)], jq[nh][:, ts(cl, 128)], SCALE)

    def stream_k0(nh):
        """Pair-0 K projection, bulk per half (emitted once wk_bf-ct0 is up)."""
        jk[nh] = pss.tile([128, 512], F32, tag="spj", name="spj", bufs=2)
        for cl in range(4):
            c = 4 * nh + cl
            for k in range(KT):
                nc.tensor.matmul(
                    jk[nh][:, ts(cl, 128)], wk_bf[:, k, ds(0, 128)],
                    xTc[c][:, ts(k, 128)],
                    start=(k == 0), stop=(k == KT - 1),
                )
        nc.vector.tensor_copy(kt_sb[0][:, ts(nh, 512)], jk[nh])

    def emit_proj(w_all, dest, ct, nh, scale):
        ps = pss.tile([128, 512], F32, tag="sp", name="sp")
        for cl in range(4):
            c = 4 * nh + cl
            for k in range(KT):
                nc.tensor.matmul(
                    ps[:, ts(cl, 128)], w_all[:, k, ts(ct, 128)],
                    xTc[c][:, ts(k, 128)],
                    start=(k == 0), stop=(k == KT - 1),
                )
        if scale == 1.0:
            nc.vector.tensor_copy(dest[ct][:, ts(nh, 512)], ps)
        else:
            nc.vector.tensor_scalar_mul(dest[ct][:, ts(nh, 512)], ps, scale)

    # expR = exp((Q/8) @ E1^T) -> DRAM scratch; exp fused into the mandatory
    # PSUM->SBUF copy.  Only the causally-needed window [lo, 1024) is
    # computed, plus a 128-col wrap chunk of filler; the whole thing goes out
    # as ONE flat DMA whose per-partition tail lands in the next DRAM row's
    # cols [0, 128) -- exactly the region the skewed read pulls as (masked)
    # upper-triangle filler.
    def emit_expr(pr, i0=0, i1=NB):
        lhs = (2 * pr, 2 * pr + 1)
        qt_t = qt[pr]
        for i in range(i0, i1):
            lo = max(0, 896 - 128 * i)
            mw = 1024 - lo                      # main window cols [lo, 1024)
            tw = mw + (128 if i < 7 else 0)     # + wrap filler
            nch = (tw + 511) // 512
            pps = [[pss.tile([128, 512], F32, tag="sp", name="sp")
                    for _ in range(nch)] for _ in range(2)]
            for hs in range(2):
                base = 64 * hs
                for c in range(nch):
                    cw = min(512, tw - 512 * c)
                    # main-window part of this chunk
                    mcw = min(cw, max(0, mw - 512 * c))
                    if mcw > 0:
                        nc.tensor.matmul(
                            pps[hs][c][:, :mcw],
                            qt_t[base:base + 64, ts(i, 128)],
                            e1t[base:base + 64, ds(lo + 512 * c, mcw)],
                            start=True, stop=True,
                        )
                    if cw > mcw:
                        # wrap filler: exp(Q @ E1^T[:, 0:128])
                        nc.tensor.matmul(
                            pps[hs][c][:, ds(mcw, cw - mcw)],
                            qt_t[base:base + 64, ts(i, 128)],
                            e1t[base:base + 64, ds(0, cw - mcw)],
                            start=True, stop=True, skip_group_check=True,
                        )
            for hs in range(2):
                qec = work.tile([128, 1024], BF16, tag=f"qec{hs}",
                                name="qec", bufs=2)
                for c in range(nch):
                    cw = min(512, tw - 512 * c)
                    # pair 0 stores exp(QE) (multiplicative combine on DVE);
                    # pair 1 stores raw QE (additive combine on the PE),
                    # copied out on the DVE to keep ACT free for exps
                    if pr == 0:
                        nc.scalar.activation(
                            qec[:, ds(512 * c, cw)], pps[hs][c][:, :cw], EXP)
                    else:
                        nc.vector.tensor_copy(
                            qec[:, ds(512 * c, cw)], pps[hs][c][:, :cw])
                nc.sync.dma_start(
                    out=bass.AP(
                        tensor=qes[lhs[hs]][:, :].tensor,
                        offset=128 * i * N + lo,
                        ap=[[N, 128], [1, tw]],
                    ),
                    in_=qec[:, :tw])

    # ---- streamed prologue: per chunk cast -> transpose -> Q0 proj ->
    # expR(0, c); the K0 projection slots in as wk's cast lands.
    for c in range(4):
        cast_transpose_chunk(c)
        stream_q0(c)
        if c == 0:
            emit_e1t()
        emit_expr(0, c, c + 1)
        if c == 1:
            # small ct0-only cast, positioned where the DVE queue would
            # otherwise idle waiting for the x stream
            nc.vector.tensor_copy(wk_bf[:, :, 0:128], wl_k[:, :, 0:128])
    stream_k0(0)

    wv_bf = persist.tile([128, KT, COLS], BF16, tag="wvb", name="wvb")

    # V: [m, col] -- one [128, 8, 256] bf16 tile; quarters emitted inside
    # pair 0's S loop to fill the PE while the softmax chain runs
    v_sb = persist.tile([128, NB, COLS], BF16, tag="vsb", name="vsb")

    def emit_v(mp):
        ps = pss.tile([128, 512], F32, tag="sp", name="sp")
        for sub in range(2):
            mb = 2 * mp + sub
            for k in range(KT):
                nc.tensor.matmul(
                    ps[:, ds(256 * sub, 256)],
                    xTc[mb][:, ts(k, 128)], wv_bf[:, k, :],
                    start=(k == 0), stop=(k == KT - 1),
                )
        nc.vector.tensor_copy(
            v_sb[:, ds(2 * mp, 2), :].rearrange("p a b -> p (a b)"), ps
        )

    wo_bf = persist.tile([128, 2, DO], BF16, tag="wob", name="wob")
    ot = [persist.tile([128, N], BF16, tag=f"ot{c}", name=f"ot{c}") for c in range(2)]
    ys = [nc.dram_tensor(f"y{pr}", [N, DO], BF16, kind="ExternalOutput")
          for pr in range(2)]

    # ---- per-pair attention, with cross-pair interleaving ----
    pts_all = {}
    state = {}

    def emit_pt(pr, i):
        """P^T = P.T @ diag(1/Z) for row block i (delayed 2 iterations).
        Pair-1 copies ride the ACT engine, which has slack in that phase."""
        pts = pts_all[pr]
        for hs in range(2):
            p_sb, dg = state.pop((pr, i, hs))
            for g in range(2):
                jn = min(i + 1 - 4 * g, 4)
                if jn <= 0:
                    break
                ps2 = pss.tile([128, 512], F32, tag="sp", name="sp")
                for jj in range(jn):
                    nc.tensor.matmul(
                        ps2[:, ts(jj, 128)],
                        p_sb[:, ts(4 * g + jj, 128)], dg,
                        start=True, stop=True,
                    )
                dst = pts[hs][g][:, 0:jn, ts(i, 128)]
                src2 = ps2[:, ds(0, 128 * jn)].rearrange(
                    "p (a b) -> p a b", a=jn)
                if (i + g) % 2 == 0:
                    nc.scalar.copy(dst, src2)
                else:
                    nc.vector.tensor_copy(dst, src2)

    rel_pf = {}

    def rel_fetch(pr, i):
        """Issue the skewed rel read for row block i (prefetchable)."""
        lhs = (2 * pr, 2 * pr + 1)
        width = 128 * (i + 1)
        rels = []
        for hs in range(2):
            rel = work.tile([128, 1024], BF16, tag=f"rel{hs}",
                            name="rel", bufs=3)
            nc.gpsimd.dma_start(out=rel[:, :width], in_=bass.AP(
                tensor=qes[lhs[hs]][:, :].tensor,
                offset=(N - 1) * (128 * i + 1),
                ap=[[N - 1, 128], [1, width]],
            ))
            if pr == 0:
                # multiplicative path: pre-mask the causal diagonal
                nc.gpsimd.tensor_tensor(
                    out=rel[:, ds(width - 128, 128)],
                    in0=rel[:, ds(width - 128, 128)], in1=tri01, op=MULT,
                )
            rels.append(rel)
        rel_pf[(pr, i)] = rels

    def s_iter(pr, i):
        """One row block of the S / expS / P / 1/Z chain for pair pr.

        Pair 0 combines exp(S)*exp(R) on the DVE (with fused row sums);
        pair 1 adds raw R and the additive causal mask into the S PSUM on
        the PE and gets P = exp(S+R+mask) plus row sums in one ACT op."""
        qt_t, kt_t = qt[pr], kt_sb[pr]
        width = 128 * (i + 1)
        nch = (width + 511) // 512
        if (pr, i) not in rel_pf:
            rel_fetch(pr, i)
        rels = rel_pf.pop((pr, i))

        add_rel = pr == 1
        spp = [[pss.tile([128, 512], F32, tag="sp", name="sp")
                for _ in range(nch)] for _ in range(2)]
        for c in range(nch):
            cw = min(512, width - 512 * c)
            for hs in range(2):
                base = 64 * hs
                nc.tensor.matmul(
                    spp[hs][c][:, :cw],
                    qt_t[base:base + 64, ts(i, 128)],
                    kt_t[base:base + 64, ds(512 * c, cw)],
                    start=True, stop=not add_rel,
                )
                if add_rel:
                    nc.tensor.matmul(
                        spp[hs][c][:, :cw], ident,
                        rels[hs][:, ds(512 * c, cw)],
                        start=False, stop=True,
                    )
            if add_rel and c == nch - 1:
                dcol = width - 128 - 512 * c
                for hs in range(2):
                    nc.tensor.matmul(
                        spp[hs][c][:, ds(dcol, 128)], ident, negtri,
                        start=False, stop=True, skip_group_check=True,
                    )
        for hs in range(2):
            p_sb = work.tile([128, 1024], BF16, tag=f"p{hs}", name="p", bufs=5)
            z = work.tile([128, 1], F32, tag=f"z{hs}", name="z")
            if add_rel:
                zc = [z]
                if nch > 1:
                    zc.append(work.tile([128, 1], F32, tag=f"z2{hs}",
                                        name="z2"))
                for c in range(nch):
                    cw = min(512, width - 512 * c)
                    nc.scalar.activation(
                        p_sb[:, ds(512 * c, cw)], spp[hs][c][:, :cw], EXP,
                        accum_out=zc[c],
                    )
                if nch > 1:
                    nc.vector.tensor_tensor(
                        out=z, in0=z, in1=zc[1], op=mybir.AluOpType.add)
            else:
                es = work.tile([128, 1024], BF16, tag=f"es{hs}",
                               name="es", bufs=2)
                for c in range(nch):
                    cw = min(512, width - 512 * c)
                    nc.scalar.activation(
                        es[:, ds(512 * c, cw)], spp[hs][c][:, :cw], EXP
                    )
                nc.vector.scalar_tensor_tensor(
                    out=p_sb[:, :width], in0=es[:, :width], scalar=1.0,
                    in1=rels[hs][:, :width], op0=MULT, op1=MULT, accum_out=z,
                )
            r = work.tile([128, 1], F32, tag=f"r{hs}", name="r")
            nc.vector.reciprocal(r, z)
            dg = work.tile([128, 128], BF16, tag=f"dg{hs}", name="dg", bufs=5)
            nc.vector.tensor_scalar_mul(dg, ident, r)
            state[(pr, i, hs)] = (p_sb, dg)

    pv_pos = {}

    def emit_pv(pr, ig, iis=(0, 1, 2, 3)):
        """O^T accumulation for row blocks 4*ig+iis of pair pr."""
        lhs = (2 * pr, 2 * pr + 1)
        pts = pts_all[pr]
        if (pr, ig) not in pv_pos:
            pv_pos[(pr, ig)] = [
                pss.tile([128, 512], F32, tag="spj", name="spj", bufs=2)
                for _ in range(2)]
        pos = pv_pos[(pr, ig)]
        for ii in iis:
            i = 4 * ig + ii
            for j in range(i + 1):
                for hs in range(2):
                    base = 64 * hs
                    nc.tensor.matmul(
                        pos[hs][base:base + 64, ts(ii, 128)],
                        v_sb[:, j, ds(64 * lhs[hs], 64)],
                        pts[hs][j // 4][:, j % 4, ts(i, 128)],
                        start=(j == 0), stop=(j == i),
                        tile_position=(0, base),
                    )
        if len(iis) != 4:
            return
        for hs in range(2):
            base = 64 * hs
            if pr == 0:
                nc.vector.tensor_copy(
                    ot[pr][base:base + 64, ds(512 * ig, 512)],
                    pos[hs][base:base + 64, :],
                )
            else:
                nc.vector.tensor_copy(
                    ot[pr][base:base + 64, ds(512 * ig, 512)],
                    pos[hs][base:base + 64, :],
                )

    def emit_yproj(pr, i):
        """Partial output projection y_pr row block i (bf16, one DMA)."""
        ysb = work.tile([128, 1024], BF16, tag=f"ypr{pr}", name="ysb",
                        bufs=2 if pr == 0 else 4)
        for oh in range(2):
            ps = pss.tile([128, 512], F32, tag="sp", name="sp")
            nc.tensor.matmul(
                ps, ot[pr][:, ts(i, 128)], wo_bf[:, pr, ds(512 * oh, 512)],
                start=True, stop=True,
            )
            if oh == 0:
                nc.scalar.copy(ysb[:, ts(oh, 512)], ps)
            else:
                nc.vector.tensor_copy(ysb[:, ts(oh, 512)], ps)
        nc.sync.dma_start(out=ys[pr][ts(i, 128), :], in_=ysb)

    # pair 0: ascending; riders stream the remaining chunks, late
    # projections, pair-0/1 expR, and V while the softmax chains run.
    pts_all[0] = [
        [ptp.tile([128, 4, 1024], BF16, tag=f"pts{hs}{g}", name="pts")
         for g in range(2)] for hs in range(2)]
    for i in range(NB):
        s_iter(0, i)
        if i < 4:
            cast_transpose_chunk(4 + i)
            stream_q0(4 + i)
        if 1 <= i <= 4:
            emit_expr(0, i + 3, i + 4)
        if i == 3:
            stream_k0(1)
            wl_v = stage.tile([128, KT, COLS], F32, tag="ld",
                              name="wldv", bufs=3)
            with tc.tile_wait_until(0.0185):
                nc.sync.dma_start(
                    out=wl_v, in_=wv.rearrange("(kt p) c -> p kt c", p=128))
        elif i == 1:
            nc.vector.tensor_copy(wq_bf[:, :, 128:256], wl_q[:, :, 128:256])
        elif i == 2:
            emit_proj(wq_bf, qt, 1, 0, SCALE)
        elif i == 5:
            emit_proj(wq_bf, qt, 1, 1, SCALE)
            nc.vector.tensor_copy(wk_bf[:, :, 128:256], wl_k[:, :, 128:256])
            wol = stage.tile([128, 2, DO], F32, tag="ld", name="wold",
                             bufs=3)
            with tc.tile_wait_until(0.0225):
                nc.sync.dma_start(
                    out=wol, in_=wo.rearrange("(ct p) c -> p ct c", p=128))
        elif i == 6:
            emit_proj(wk_bf, kt_sb, 1, 0, 1.0)
            emit_proj(wk_bf, kt_sb, 1, 1, 1.0)
            nc.vector.tensor_copy(wv_bf, wl_v)
            emit_expr(1, 7, 8)
        elif i == 7:
            emit_expr(1, 6, 7)
            emit_expr(1, 5, 6)
            emit_expr(1, 4, 5)
            emit_v(0)
            emit_v(1)
        if i >= 3:
            emit_pt(0, i - 3)
    nc.vector.tensor_copy(wo_bf, wol)
    for ii in (NB - 3, NB - 2, NB - 1):
        emit_pt(0, ii)

    # pair 1: DESCENDING row blocks; riders = pair-0 PV / y projection.
    # PT delay runs on processed-count; pv(1,1) fires once PTs for rows
    # 4..7 are out, its yproj rides the remaining short iterations, and
    # the tail is only the cheap low rows' chain + pv(1,0) + yproj(1,0..3).
    pts_all[1] = [
        [ptp.tile([128, 4, 1024], BF16, tag=f"pts{hs}{g}", name="pts")
         for g in range(2)] for hs in range(2)]
    order = list(range(NB - 1, -1, -1))
    rel_fetch(1, order[0])
    for t, i in enumerate(order):
        s_iter(1, i)
        if t + 1 < NB:
            rel_fetch(1, order[t + 1])
        if t < 4:
            emit_expr(1, 3 - t, 4 - t)
        if t == 0:
            emit_v(2)
            emit_v(3)
        elif t == 1:
            emit_pv(0, 0)
        elif t == 2:
            emit_pv(0, 1)
        elif t in (3, 4, 5):
            emit_yproj(0, 2 * (t - 3))
            emit_yproj(0, 2 * (t - 3) + 1)
        elif t == 6:
            emit_yproj(0, 6)
            emit_yproj(0, 7)
        if t >= 3:
            emit_pt(1, order[t - 3])
        if t == 6:
            emit_pv(1, 1, iis=(3, 2, 1, 0))
        elif t == 7:
            emit_yproj(1, 4)
            emit_yproj(1, 5)
            emit_yproj(1, 6)
            emit_yproj(1, 7)
    for t in (NB - 3, NB - 2, NB - 1):
        emit_pt(1, order[t])
    emit_pv(1, 0, iis=(3, 2, 1, 0))
    for i in range(4):
        emit_yproj(1, i)

    ctx.close()


_NC_CACHE = None


def _get_nc():
    global _NC_CACHE
    if _NC_CACHE is None:
        nc = bacc.Bacc(
            "TRN2", target_bir_lowering=False, debug=False, num_devices=8
        )
        with tile.TileContext(nc) as tc:
            _body(tc)
        nc.compile()
        _NC_CACHE = nc
    return _NC_CACHE


def make_in_maps(x, E_rel, Wq, Wk, Wv, Wo):
    in_maps = []
    for c in range(8):
        b, g = c // 4, c % 4
        cols = slice(COLS * g, COLS * (g + 1))
        in_maps.append({
            "xb": np.ascontiguousarray(x[b], dtype=np.float32),
            "wq": np.ascontiguousarray(Wq[:, cols], dtype=np.float32),
            "wk": np.ascontiguousarray(Wk[:, cols], dtype=np.float32),
            "wv": np.ascontiguousarray(Wv[:, cols], dtype=np.float32),
            "wo": np.ascontiguousarray(Wo[cols, :], dtype=np.float32),
            "e1": np.ascontiguousarray(E_rel[:N], dtype=np.float32),
        })
    return in_maps


def combine(results, bo):
    parts = [
        np.asarray(results[c]["y0"], dtype=np.float32)
        + np.asarray(results[c]["y1"], dtype=np.float32)
        for c in range(8)
    ]
    out0 = parts[0] + parts[1] + parts[2] + parts[3] + bo.astype(np.float32)
    out1 = parts[4] + parts[5] + parts[6] + parts[7] + bo.astype(np.float32)
    return np.stack([out0, out1]).astype(np.float32)


def kernel(x, E_rel, mask, Wq, Wk, Wv, Wo, bo, **_):
    nc = _get_nc()
    in_maps = make_in_maps(
        np.asarray(x), np.asarray(E_rel), np.asarray(Wq), np.asarray(Wk),
        np.asarray(Wv), np.asarray(Wo),
    )
    res = run_bass_kernel_spmd(nc, in_maps, list(range(8)))
    return combine(res.results, np.asarray(bo))


# revision 78
# speedup vs baseline: 1.0092x; 1.0092x over previous
"""Multi-head attention with relative position bias (music-transformer skew)
on 8 Trainium2 NeuronCores.

Sharding: batch x head-group. Core c handles batch b = c//4 and heads
4*(c%4) .. 4*(c%4)+3 (tensor-parallel split of the 16 heads / W column dims,
Wo row-parallel). Partial outputs are summed on the host (the all-reduce
equivalent), bias added on the host.

Per-core pipeline (bf16 matmuls, fp32 PSUM accumulation):
  x^T via PE transposes -> Q^T,K^T ([col,n]) and V ([m,col]) projections.
  Per head pair, one software-pipelined loop over row blocks i:
    expR = exp((Q/8) @ E1^T) -> DRAM (exp fused into the mandatory
    PSUM->SBUF copy) as ONE flat DMA per (i,hs) whose per-partition tail
    spills into the next DRAM row's low columns (the filler the skewed
    read needs); skew-read back (stride N-1 access pattern) as relE with
    the causal diagonal pre-masked by a 0/1 triangle;
    expS = exp((Q/8) @ K^T) straight out of PSUM;
    P = expS * relE with row sums Z fused into the same DVE op;
    P^T = P.T @ diag(1/Z) on the PE (normalization for free), delayed two
    iterations so the PE never stalls on the softmax chain;
  then O^T accumulates V-side (two heads in complementary PE column groups)
  and y = O^T.T @ Wo (bf16 partials, summed on the host).
  Pair 1 walks its row blocks in DESCENDING order so the final softmax
  chains are the cheap short rows and the O^T/y work for its high row
  blocks overlaps the remaining short iterations.
"""

import numpy as np

import concourse.bass as bass
import concourse.tile as tile
from concourse import bacc, mybir
from concourse.bass import ds, ts
from concourse.bass_utils import run_bass_kernel_spmd
from concourse.masks import make_identity

B, N, D, H, DH, DO = 2, 1024, 1024, 16, 64, 1024
HPC = 4              # heads per core
COLS = HPC * DH      # 256 projection columns per core
NB = N // 128        # 8 row blocks
KT = D // 128        # 8 contraction tiles
F32 = mybir.dt.float32
BF16 = mybir.dt.bfloat16
SCALE = 1.0 / np.sqrt(DH)
EXP = mybir.ActivationFunctionType.Exp
MULT = mybir.AluOpType.mult


def _body(tc):
    nc = tc.nc
    xb = nc.dram_tensor("xb", [N, D], F32, kind="ExternalInput")
    wq = nc.dram_tensor("wq", [D, COLS], F32, kind="ExternalInput")
    wk = nc.dram_tensor("wk", [D, COLS], F32, kind="ExternalInput")
    wv = nc.dram_tensor("wv", [D, COLS], F32, kind="ExternalInput")
    wo = nc.dram_tensor("wo", [COLS, DO], F32, kind="ExternalInput")
    e1 = nc.dram_tensor("e1", [N, DH], F32, kind="ExternalInput")
    qes = [nc.dram_tensor(f"qes{h}", [N, N], BF16) for h in range(HPC)]

    from contextlib import ExitStack
    ctx = ExitStack()
    singles = ctx.enter_context(tc.tile_pool(name="singles", bufs=1))
    persist = ctx.enter_context(tc.tile_pool(name="persist", bufs=1))
    stage = ctx.enter_context(tc.tile_pool(name="stage", bufs=2))
    work = ctx.enter_context(tc.tile_pool(name="work", bufs=2))
    ptp = ctx.enter_context(tc.tile_pool(name="ptp", bufs=1))
    pss = ctx.enter_context(tc.tile_pool(name="pss", bufs=6, space="PSUM"))

    ident = singles.tile([128, 128], BF16, tag="ident", name="ident")
    make_identity(nc, ident)

    # ---- x: 8 chunked loads (Pool SWDGE), pipelined ACT casts, PE transposes.
    # Staging layout: xTc[c][:, k*128:(k+1)*128] = x^T k-tile of n-chunk c,
    # i.e. x[n-block c, :].T laid out k-major.  Consumers index per chunk.
    xTc = [persist.tile([128, D], BF16, tag=f"xTc{c}", name=f"xTc{c}")
           for c in range(NB)]

    # weight/E loads ride the DVE HWDGE queue; the wk issue deliberately
    # sits AFTER the wq-ct0 cast in the DVE queue so its transfer
    # interleaves behind the first x chunks on the (serial) DMA engines.
    wl_q = stage.tile([128, KT, COLS], F32, tag="ld", name="wldq", bufs=3)
    wqr = wq.rearrange("(kt p) c -> p kt c", p=128)
    nc.sync.dma_start(out=wl_q[:, :, 0:128], in_=wqr[:, :, 0:128])
    wq_bf = persist.tile([128, KT, COLS], BF16, tag="wqb", name="wqb")
    nc.vector.tensor_copy(wq_bf[:, :, 0:128], wl_q[:, :, 0:128])
    with tc.tile_wait_until(0.0200):
        nc.sync.dma_start(out=wl_q[:, :, 128:256], in_=wqr[:, :, 128:256])
    wl_k = stage.tile([128, KT, COLS], F32, tag="ld", name="wldk", bufs=3)
    el = stage.tile([128, 8, DH], F32, tag="lde", name="eld", bufs=1)

    # x chunks, e1 and wk all ride the Pool SWDGE queue: descriptor
    # generation is serial (~1us each), which naturally staggers their
    # transfer-start times into the intended interleave on the (serial)
    # DMA engines: wq0 | x0 | e1 | x1 x2 x3 | wk | x4..x7.  The late
    # chunks are pinned so the first qes writes / skew reads can slot in.
    xls = []
    xpin = {4: 0.0125, 5: 0.0140, 6: 0.0155, 7: 0.0170}
    for c in range(NB):
        xl = stage.tile([128, D], F32, tag="xld", name="xld", bufs=4)
        with tc.tile_wait_until(xpin.get(c, 0), enable=c in xpin):
            nc.gpsimd.dma_start(
                out=xl, in_=xb.rearrange("(nt p) d -> p nt d", p=128)[:, c, :])
        xls.append(xl)
        if c == 0:
            nc.gpsimd.dma_start(
                out=el, in_=e1.rearrange("(nt p) d -> p nt d", p=128))
        elif c == 1:
            nc.gpsimd.dma_start(
                out=wl_k, in_=wk.rearrange("(kt p) c -> p kt c", p=128))

    # 0/1 lower-triangular (causal keep) mask, bf16
    tri01 = singles.tile([128, 128], BF16, tag="tri01", name="tri01")
    nc.gpsimd.memset(tri01, 1.0)
    nc.gpsimd.affine_select(
        out=tri01, in_=tri01, compare_op=mybir.AluOpType.is_ge,
        fill=0.0, base=0, pattern=[[-1, 128]], channel_multiplier=1,
    )
    # additive causal mask: 0 on lower incl diag, -1e9 strictly above
    negtri = singles.tile([128, 128], BF16, tag="negtri", name="negtri")
    nc.gpsimd.memset(negtri, 0.0)
    nc.gpsimd.affine_select(
        out=negtri, in_=negtri, compare_op=mybir.AluOpType.is_ge,
        fill=-1e9, base=0, pattern=[[-1, 128]], channel_multiplier=1,
    )

    e1t = singles.tile([128, N], BF16, tag="e1t", name="e1t")

    def emit_e1t():
        # E1^T [64, 1024] bf16, duplicated to partitions 64..127
        ec = stage.tile([128, 8, DH], BF16, tag="ec", name="ec", bufs=1)
        nc.vector.tensor_copy(ec, el)
        for half in range(2):
            ps = pss.tile([64, 512], BF16, tag="sp", name="sp")
            for q in range(4):
                nc.tensor.transpose(
                    ps[:, ts(q, 128)], ec[:, 4 * half + q, :], ident)
            nc.vector.tensor_copy(e1t[0:64, ds(512 * half, 512)], ps)
        nc.sync.dma_start(out=e1t[64:128, :], in_=e1t[0:64, :])

    qt = [persist.tile([128, N], BF16, tag=f"qt{c}", name=f"qt{c}") for c in range(2)]
    kt_sb = [persist.tile([128, N], BF16, tag=f"kt{c}", name=f"kt{c}") for c in range(2)]
    wk_bf = persist.tile([128, KT, COLS], BF16, tag="wkb", name="wkb")

    # held PSUM banks (tag spj) for the streamed pair-0 projections; the
    # same two banks are reused by pv_pos later in the kernel.
    jq = [None, None]
    jk = [None, None]

    def cast_transpose_chunk(c):
        xc = stage.tile([128, D], BF16, tag="xcc", name="xcc", bufs=3)
        if c % 2 == 0:
            nc.vector.tensor_copy(xc, xls[c])
        else:
            nc.scalar.copy(xc, xls[c])
        for h in range(2):
            ps = pss.tile([128, 512], BF16, tag="sp", name="sp")
            for k in range(4):
                nc.tensor.transpose(
                    ps[:, ts(k, 128)], xc[:, ts(4 * h + k, 128)], ident)
            nc.vector.tensor_copy(xTc[c][:, ts(h, 512)], ps)

    def stream_q0(c):
        """Pair-0 Q projection column c, copied out (scaled) per chunk."""
        nh, cl = c // 4, c % 4
        if cl == 0:
            jq[nh] = pss.tile([128, 512], F32, tag="spj", name="spj", bufs=2)
        for k in range(KT):
            nc.tensor.matmul(
                jq[nh][:, ts(cl, 128)], wq_bf[:, k, ds(0, 128)],
                xTc[c][:, ts(k, 128)],
                start=(k == 0), stop=(k == KT - 1),
            )
        nc.vector.tensor_scalar_mul(
            qt[0][:, ts(c, 128)], jq[nh][:, ts(cl, 128)], SCALE)

    def stream_k0(nh):
        """Pair-0 K projection, bulk per half (emitted once wk_bf-ct0 is up)."""
        jk[nh] = pss.tile([128, 512], F32, tag="spj", name="spj", bufs=2)
        for cl in range(4):
            c = 4 * nh + cl
            for k in range(KT):
                nc.tensor.matmul(
                    jk[nh][:, ts(cl, 128)], wk_bf[:, k, ds(0, 128)],
                    xTc[c][:, ts(k, 128)],
                    start=(k == 0), stop=(k == KT - 1),
                )
        nc.vector.tensor_copy(kt_sb[0][:, ts(nh, 512)], jk[nh])

    def emit_proj(w_all, dest, ct, nh, scale):
        ps = pss.tile([128, 512], F32, tag="sp", name="sp")
        for cl in range(4):
            c = 4 * nh + cl
            for k in range(KT):
                nc.tensor.matmul(
                    ps[:, ts(cl, 128)], w_all[:, k, ts(ct, 128)],
                    xTc[c][:, ts(k, 128)],
                    start=(k == 0), stop=(k == KT - 1),
                )
        if scale == 1.0:
            nc.vector.tensor_copy(dest[ct][:, ts(nh, 512)], ps)
        else:
            nc.vector.tensor_scalar_mul(dest[ct][:, ts(nh, 512)], ps, scale)

    # expR = exp((Q/8) @ E1^T) -> DRAM scratch; exp fused into the mandatory
    # PSUM->SBUF copy.  Only the causally-needed window [lo, 1024) is
    # computed, plus a 128-col wrap chunk of filler; the whole thing goes out
    # as ONE flat DMA whose per-partition tail lands in the next DRAM row's
    # cols [0, 128) -- exactly the region the skewed read pulls as (masked)
    # upper-triangle filler.
    def emit_expr(pr, i0=0, i1=NB):
        lhs = (2 * pr, 2 * pr + 1)
        qt_t = qt[pr]
        for i in range(i0, i1):
            lo = max(0, 896 - 128 * i)
            mw = 1024 - lo                      # main window cols [lo, 1024)
            tw = mw + (128 if i < 7 else 0)     # + wrap filler
            nch = (tw + 511) // 512
            pps = [[pss.tile([128, 512], F32, tag="sp", name="sp")
                    for _ in range(nch)] for _ in range(2)]
            for hs in range(2):
                base = 64 * hs
                for c in range(nch):
                    cw = min(512, tw - 512 * c)
                    # main-window part of this chunk
                    mcw = min(cw, max(0, mw - 512 * c))
                    if mcw > 0:
                        nc.tensor.matmul(
                            pps[hs][c][:, :mcw],
                            qt_t[base:base + 64, ts(i, 128)],
                            e1t[base:base + 64, ds(lo + 512 * c, mcw)],
                            start=True, stop=True,
                        )
                    if cw > mcw:
                        # wrap filler: exp(Q @ E1^T[:, 0:128])
                        nc.tensor.matmul(
                            pps[hs][c][:, ds(mcw, cw - mcw)],
                            qt_t[base:base + 64, ts(i, 128)],
                            e1t[base:base + 64, ds(0, cw - mcw)],
                            start=True, stop=True, skip_group_check=True,
                        )
            for hs in range(2):
                qec = work.tile([128, 1024], BF16, tag=f"qec{hs}",
                                name="qec", bufs=2)
                for c in range(nch):
                    cw = min(512, tw - 512 * c)
                    # pair 0 stores exp(QE) (multiplicative combine on DVE);
                    # pair 1 stores raw QE (additive combine on the PE),
                    # copied out on the DVE to keep ACT free for exps
                    if pr == 0:
                        nc.scalar.activation(
                            qec[:, ds(512 * c, cw)], pps[hs][c][:, :cw], EXP)
                    else:
                        nc.vector.tensor_copy(
                            qec[:, ds(512 * c, cw)], pps[hs][c][:, :cw])
                nc.sync.dma_start(
                    out=bass.AP(
                        tensor=qes[lhs[hs]][:, :].tensor,
                        offset=128 * i * N + lo,
                        ap=[[N, 128], [1, tw]],
                    ),
                    in_=qec[:, :tw])

    # ---- streamed prologue: per chunk cast -> transpose -> Q0 proj ->
    # expR(0, c); the K0 projection slots in as wk's cast lands.
    for c in range(4):
        cast_transpose_chunk(c)
        stream_q0(c)
        if c == 0:
            emit_e1t()
        emit_expr(0, c, c + 1)
        if c == 1:
            # small ct0-only cast, positioned where the DVE queue would
            # otherwise idle waiting for the x stream
            nc.vector.tensor_copy(wk_bf[:, :, 0:128], wl_k[:, :, 0:128])
    stream_k0(0)

    wv_bf = persist.tile([128, KT, COLS], BF16, tag="wvb", name="wvb")

    # V: [m, col] -- one [128, 8, 256] bf16 tile; quarters emitted inside
    # pair 0's S loop to fill the PE while the softmax chain runs
    v_sb = persist.tile([128, NB, COLS], BF16, tag="vsb", name="vsb")

    def emit_v(mp):
        ps = pss.tile([128, 512], F32, tag="sp", name="sp")
        for sub in range(2):
            mb = 2 * mp + sub
            for k in range(KT):
                nc.tensor.matmul(
                    ps[:, ds(256 * sub, 256)],
                    xTc[mb][:, ts(k, 128)], wv_bf[:, k, :],
                    start=(k == 0), stop=(k == KT - 1),
                )
        nc.vector.tensor_copy(
            v_sb[:, ds(2 * mp, 2), :].rearrange("p a b -> p (a b)"), ps
        )

    wo_bf = persist.tile([128, 2, DO], BF16, tag="wob", name="wob")
    ot = [persist.tile([128, N], BF16, tag=f"ot{c}", name=f"ot{c}") for c in range(2)]
    ys = [nc.dram_tensor(f"y{pr}", [N, DO], BF16, kind="ExternalOutput")
          for pr in range(2)]

    # ---- per-pair attention, with cross-pair interleaving ----
    pts_all = {}
    state = {}

    def emit_pt(pr, i):
        """P^T = P.T @ diag(1/Z) for row block i (delayed 2 iterations).
        Pair-1 copies ride the ACT engine, which has slack in that phase."""
        pts = pts_all[pr]
        for hs in range(2):
            p_sb, dg = state.pop((pr, i, hs))
            for g in range(2):
                jn = min(i + 1 - 4 * g, 4)
                if jn <= 0:
                    break
                ps2 = pss.tile([128, 512], F32, tag="sp", name="sp")
                for jj in range(jn):
                    nc.tensor.matmul(
                        ps2[:, ts(jj, 128)],
                        p_sb[:, ts(4 * g + jj, 128)], dg,
                        start=True, stop=True,
                    )
                dst = pts[hs][g][:, 0:jn, ts(i, 128)]
                src2 = ps2[:, ds(0, 128 * jn)].rearrange(
                    "p (a b) -> p a b", a=jn)
                if (i + g) % 2 == 0:
                    nc.scalar.copy(dst, src2)
                else:
                    nc.vector.tensor_copy(dst, src2)

    rel_pf = {}

    def rel_fetch(pr, i):
        """Issue the skewed rel read for row block i (prefetchable)."""
        lhs = (2 * pr, 2 * pr + 1)
        width = 128 * (i + 1)
        rels = []
        for hs in range(2):
            rel = work.tile([128, 1024], BF16, tag=f"rel{hs}",
                            name="rel", bufs=3)
            nc.gpsimd.dma_start(out=rel[:, :width], in_=bass.AP(
                tensor=qes[lhs[hs]][:, :].tensor,
                offset=(N - 1) * (128 * i + 1),
                ap=[[N - 1, 128], [1, width]],
            ))
            if pr == 0:
                # multiplicative path: pre-mask the causal diagonal
                nc.gpsimd.tensor_tensor(
                    out=rel[:, ds(width - 128, 128)],
                    in0=rel[:, ds(width - 128, 128)], in1=tri01, op=MULT,
                )
            rels.append(rel)
        rel_pf[(pr, i)] = rels

    def s_iter(pr, i):
        """One row block of the S / expS / P / 1/Z chain for pair pr.

        Pair 0 combines exp(S)*exp(R) on the DVE (with fused row sums);
        pair 1 adds raw R and the additive causal mask into the S PSUM on
        the PE and gets P = exp(S+R+mask) plus row sums in one ACT op."""
        qt_t, kt_t = qt[pr], kt_sb[pr]
        width = 128 * (i + 1)
        nch = (width + 511) // 512
        if (pr, i) not in rel_pf:
            rel_fetch(pr, i)
        rels = rel_pf.pop((pr, i))

        add_rel = pr == 1
        spp = [[pss.tile([128, 512], F32, tag="sp", name="sp")
                for _ in range(nch)] for _ in range(2)]
        for c in range(nch):
            cw = min(512, width - 512 * c)
            for hs in range(2):
                base = 64 * hs
                nc.tensor.matmul(
                    spp[hs][c][:, :cw],
                    qt_t[base:base + 64, ts(i, 128)],
                    kt_t[base:base + 64, ds(512 * c, cw)],
                    start=True, stop=not add_rel,
                )
                if add_rel:
                    nc.tensor.matmul(
                        spp[hs][c][:, :cw], ident,
                        rels[hs][:, ds(512 * c, cw)],
                        start=False, stop=True,
                    )
            if add_rel and c == nch - 1:
                dcol = width - 128 - 512 * c
                for hs in range(2):
                    nc.tensor.matmul(
                        spp[hs][c][:, ds(dcol, 128)], ident, negtri,
                        start=False, stop=True, skip_group_check=True,
                    )
        for hs in range(2):
            p_sb = work.tile([128, 1024], BF16, tag=f"p{hs}", name="p", bufs=5)
            z = work.tile([128, 1], F32, tag=f"z{hs}", name="z")
            if add_rel:
                zc = [z]
                if nch > 1:
                    zc.append(work.tile([128, 1], F32, tag=f"z2{hs}",
                                        name="z2"))
                for c in range(nch):
                    cw = min(512, width - 512 * c)
                    nc.scalar.activation(
                        p_sb[:, ds(512 * c, cw)], spp[hs][c][:, :cw], EXP,
                        accum_out=zc[c],
                    )
                if nch > 1:
                    nc.vector.tensor_tensor(
                        out=z, in0=z, in1=zc[1], op=mybir.AluOpType.add)
            else:
                es = work.tile([128, 1024], BF16, tag=f"es{hs}",
                               name="es", bufs=2)
                for c in range(nch):
                    cw = min(512, width - 512 * c)
                    nc.scalar.activation(
                        es[:, ds(512 * c, cw)], spp[hs][c][:, :cw], EXP
                    )
                nc.vector.scalar_tensor_tensor(
                    out=p_sb[:, :width], in0=es[:, :width], scalar=1.0,
                    in1=rels[hs][:, :width], op0=MULT, op1=MULT, accum_out=z,
                )
            r = work.tile([128, 1], F32, tag=f"r{hs}", name="r")
            nc.vector.reciprocal(r, z)
            dg = work.tile([128, 128], BF16, tag=f"dg{hs}", name="dg", bufs=5)
            nc.vector.tensor_scalar_mul(dg, ident, r)
            state[(pr, i, hs)] = (p_sb, dg)

    pv_pos = {}

    def emit_pv(pr, ig, iis=(0, 1, 2, 3)):
        """O^T accumulation for row blocks 4*ig+iis of pair pr."""
        lhs = (2 * pr, 2 * pr + 1)
        pts = pts_all[pr]
        if (pr, ig) not in pv_pos:
            pv_pos[(pr, ig)] = [
                pss.tile([128, 512], F32, tag="spj", name="spj", bufs=2)
                for _ in range(2)]
        pos = pv_pos[(pr, ig)]
        for ii in iis:
            i = 4 * ig + ii
            for j in range(i + 1):
                for hs in range(2):
                    base = 64 * hs
                    nc.tensor.matmul(
                        pos[hs][base:base + 64, ts(ii, 128)],
                        v_sb[:, j, ds(64 * lhs[hs], 64)],
                        pts[hs][j // 4][:, j % 4, ts(i, 128)],
                        start=(j == 0), stop=(j == i),
                        tile_position=(0, base),
                    )
        if len(iis) != 4:
            return
        for hs in range(2):
            base = 64 * hs
            if pr == 0:
                nc.vector.tensor_copy(
                    ot[pr][base:base + 64, ds(512 * ig, 512)],
                    pos[hs][base:base + 64, :],
                )
            else:
                nc.vector.tensor_copy(
                    ot[pr][base:base + 64, ds(512 * ig, 512)],
                    pos[hs][base:base + 64, :],
                )

    def emit_yproj(pr, i):
        """Partial output projection y_pr row block i (bf16, one DMA)."""
        ysb = work.tile([128, 1024], BF16, tag=f"ypr{pr}", name="ysb",
                        bufs=2 if pr == 0 else 4)
        for oh in range(2):
            ps = pss.tile([128, 512], F32, tag="sp", name="sp")
            nc.tensor.matmul(
                ps, ot[pr][:, ts(i, 128)], wo_bf[:, pr, ds(512 * oh, 512)],
                start=True, stop=True,
            )
            if oh == 0:
                nc.scalar.copy(ysb[:, ts(oh, 512)], ps)
            else:
                nc.vector.tensor_copy(ysb[:, ts(oh, 512)], ps)
        nc.sync.dma_start(out=ys[pr][ts(i, 128), :], in_=ysb)

    # pair 0: ascending; riders stream the remaining chunks, late
    # projections, pair-0/1 expR, and V while the softmax chains run.
    pts_all[0] = [
        [ptp.tile([128, 4, 1024], BF16, tag=f"pts{hs}{g}", name="pts")
         for g in range(2)] for hs in range(2)]
    for i in range(NB):
        s_iter(0, i)
        if i < 4:
            cast_transpose_chunk(4 + i)
            stream_q0(4 + i)
        if 1 <= i <= 4:
            emit_expr(0, i + 3, i + 4)
        if i == 3:
            stream_k0(1)
            wl_v = stage.tile([128, KT, COLS], F32, tag="ld",
                              name="wldv", bufs=3)
            with tc.tile_wait_until(0.0185):
                nc.sync.dma_start(
                    out=wl_v, in_=wv.rearrange("(kt p) c -> p kt c", p=128))
        elif i == 4:
            nc.vector.tensor_copy(wq_bf[:, :, 128:256], wl_q[:, :, 128:256])
        elif i == 5:
            emit_proj(wq_bf, qt, 1, 0, SCALE)
            emit_proj(wq_bf, qt, 1, 1, SCALE)
            nc.vector.tensor_copy(wk_bf[:, :, 128:256], wl_k[:, :, 128:256])
            wol = stage.tile([128, 2, DO], F32, tag="ld", name="wold",
                             bufs=3)
            with tc.tile_wait_until(0.0225):
                nc.sync.dma_start(
                    out=wol, in_=wo.rearrange("(ct p) c -> p ct c", p=128))
        elif i == 6:
            emit_proj(wk_bf, kt_sb, 1, 0, 1.0)
            emit_proj(wk_bf, kt_sb, 1, 1, 1.0)
            nc.vector.tensor_copy(wv_bf, wl_v)
            emit_expr(1, 7, 8)
        elif i == 7:
            emit_expr(1, 6, 7)
            emit_expr(1, 5, 6)
            emit_expr(1, 4, 5)
            emit_v(0)
            emit_v(1)
        if i >= 3:
            emit_pt(0, i - 3)
    nc.vector.tensor_copy(wo_bf, wol)
    for ii in (NB - 3, NB - 2, NB - 1):
        emit_pt(0, ii)

    # pair 1: DESCENDING row blocks; riders = pair-0 PV / y projection.
    # PT delay runs on processed-count; pv(1,1) fires once PTs for rows
    # 4..7 are out, its yproj rides the remaining short iterations, and
    # the tail is only the cheap low rows' chain + pv(1,0) + yproj(1,0..3).
    pts_all[1] = [
        [ptp.tile([128, 4, 1024], BF16, tag=f"pts{hs}{g}", name="pts")
         for g in range(2)] for hs in range(2)]
    order = list(range(NB - 1, -1, -1))
    rel_fetch(1, order[0])
    for t, i in enumerate(order):
        s_iter(1, i)
        if t + 1 < NB:
            rel_fetch(1, order[t + 1])
        if t < 4:
            emit_expr(1, 3 - t, 4 - t)
        if t == 0:
            emit_v(2)
            emit_v(3)
        elif t == 1:
            emit_pv(0, 0)
        elif t == 2:
            emit_pv(0, 1)
        elif t in (3, 4, 5):
            emit_yproj(0, 2 * (t - 3))
            emit_yproj(0, 2 * (t - 3) + 1)
        elif t == 6:
            emit_yproj(0, 6)
            emit_yproj(0, 7)
        if t >= 3:
            emit_pt(1, order[t - 3])
        if t == 6:
            emit_pv(1, 1, iis=(3, 2, 1, 0))
        elif t == 7:
            emit_yproj(1, 4)
            emit_yproj(1, 5)
            emit_yproj(1, 6)
            emit_yproj(1, 7)
    for t in (NB - 3, NB - 2, NB - 1):
        emit_pt(1, order[t])
    emit_pv(1, 0, iis=(3, 2, 1, 0))
    for i in range(4):
        emit_yproj(1, i)

    ctx.close()


_NC_CACHE = None


def _get_nc():
    global _NC_CACHE
    if _NC_CACHE is None:
        nc = bacc.Bacc(
            "TRN2", target_bir_lowering=False, debug=False, num_devices=8
        )
        with tile.TileContext(nc) as tc:
            _body(tc)
        nc.compile()
        _NC_CACHE = nc
    return _NC_CACHE


def make_in_maps(x, E_rel, Wq, Wk, Wv, Wo):
    in_maps = []
    for c in range(8):
        b, g = c // 4, c % 4
        cols = slice(COLS * g, COLS * (g + 1))
        in_maps.append({
            "xb": np.ascontiguousarray(x[b], dtype=np.float32),
            "wq": np.ascontiguousarray(Wq[:, cols], dtype=np.float32),
            "wk": np.ascontiguousarray(Wk[:, cols], dtype=np.float32),
            "wv": np.ascontiguousarray(Wv[:, cols], dtype=np.float32),
            "wo": np.ascontiguousarray(Wo[cols, :], dtype=np.float32),
            "e1": np.ascontiguousarray(E_rel[:N], dtype=np.float32),
        })
    return in_maps


def combine(results, bo):
    parts = [
        np.asarray(results[c]["y0"], dtype=np.float32)
        + np.asarray(results[c]["y1"], dtype=np.float32)
        for c in range(8)
    ]
    out0 = parts[0] + parts[1] + parts[2] + parts[3] + bo.astype(np.float32)
    out1 = parts[4] + parts[5] + parts[6] + parts[7] + bo.astype(np.float32)
    return np.stack([out0, out1]).astype(np.float32)


def kernel(x, E_rel, mask, Wq, Wk, Wv, Wo, bo, **_):
    nc = _get_nc()
    in_maps = make_in_maps(
        np.asarray(x), np.asarray(E_rel), np.asarray(Wq), np.asarray(Wk),
        np.asarray(Wv), np.asarray(Wo),
    )
    res = run_bass_kernel_spmd(nc, in_maps, list(range(8)))
    return combine(res.results, np.asarray(bo))
